# revision 6
# baseline (speedup 1.0000x reference)
"""Trainium2 Bass kernel for nn_NodeModel (gnn_message_passing).

Reference computation:
    agg = segment_sum(edge_attr, edge_index[0], N)   # [N, 64]
    h   = relu(concat([x, agg], 1) @ W1 + b1)        # [N, 256]
    out = h @ W2 + b2                                # [N, 64]
(u and batch are unused by the reference.)

Strategy (8 cores, graph-parallel):
  * Host assigns nodes to 64-node "windows" (208 windows/core), balancing
    edge counts per window with a greedy heap so every window holds <= 1024
    edges.  Edges are routed to the window owning their row endpoint and
    padded to exactly 8 chunks of 128 edges per window.
  * Device scatter-add per window: for each 128-edge chunk, build a one-hot
    [128 edges, 64 nodes] matrix on the Vector engine (iota == rel compare)
    and accumulate  aggT[64f, 64n] += edge_chunk.T @ onehot  on the PE.
    Everything stays feature-major ("transposed") so no on-device transposes
    are needed anywhere.
  * MLP runs on 512-node supertiles:  hT = relu(W1.T @ [xT; aggT] + b1),
    outT = W2.T @ hT + b2, all with features on partitions.
  * Edge data travels as bf16 (halves the dominant DMA traffic); the MLP
    runs in float32r (full fp32 layout, fast PE mode, ~1e-4 matmul error).
"""

import os
import sys
import heapq

for _p in ("/opt/trn_rl_repo", "/root/.axon_site/_ro/trn_rl_repo"):
    if os.path.isdir(_p) and _p not in sys.path:
        sys.path.insert(0, _p)

import numpy as np
import ml_dtypes
from contextlib import ExitStack

import concourse.bass as bass
import concourse.tile as tile
from concourse import bacc, mybir
from concourse.bass_utils import run_bass_kernel_spmd

F32 = mybir.dt.float32
F32R = mybir.dt.float32r
BF16 = mybir.dt.bfloat16

NCORES = 8
D = 64            # feature dim
H = 256           # hidden dim
O = 64            # output dim
W = 64            # nodes per window
CHUNK = 128       # edges per chunk (PE contraction dim)
CHUNKS = 8        # chunks per window
CAP = CHUNK * CHUNKS   # 1024 edge slots per window
G = 4             # windows per edge-DMA group / onehot op
ST = 512          # MLP supertile (nodes); ST == 8 * W


class Cfg:
    def __init__(self, n_nodes, n_edges):
        wpc = max(
            (n_nodes + NCORES * W - 1) // (NCORES * W),
            int(np.ceil(n_edges * 1.07 / (CAP * NCORES))),
        )
        stw = ST // W
        wpc = ((wpc + stw - 1) // stw) * stw      # supertile-align
        self.WPC = wpc                            # windows per core
        self.NPC = W * wpc                        # node slots per core
        self.NWIN = NCORES * wpc
        self.NGRP = wpc // G                      # edge DMA groups per core
        self.NST = self.NPC // ST                 # MLP supertiles per core


# ----------------------------------------------------------------- host pack

def _assign_nodes(row, n_nodes, cfg):
    """slot_of_node: balanced node->slot map; perm: slot->node (-1 = pad)."""
    deg = np.bincount(row, minlength=n_nodes)
    order = np.argsort(-deg, kind="stable")
    nwin = cfg.NWIN
    heap = [(0, w) for w in range(nwin)]
    counts = np.zeros(nwin, np.int64)     # nodes per window
    loads = np.zeros(nwin, np.int64)      # edges per window
    slot_of_node = np.full(n_nodes, -1, np.int64)
    for n in order:
        d = int(deg[n])
        while True:
            load, w = heapq.heappop(heap)
            if counts[w] < W:
                break
        slot_of_node[n] = w * W + counts[w]
        counts[w] += 1
        loads[w] = load + d
        if counts[w] < W:
            heapq.heappush(heap, (loads[w], w))
    assert loads.max() <= CAP, f"window overflow: {loads.max()} > {CAP}"
    perm = np.full(nwin * W, -1, np.int64)
    perm[slot_of_node] = np.arange(n_nodes)
    return slot_of_node, perm


def _pack(x, edge_index, edge_attr, W1, b1, W2, b2, cfg):
    n_nodes = x.shape[0]
    n_edges = edge_attr.shape[0]
    row = np.asarray(edge_index[0], np.int64)
    slot_of_node, perm = _assign_nodes(row, n_nodes, cfg)

    # ---- node features, transposed + permuted, split per core
    slots = np.zeros((cfg.NWIN * W, D), np.float32)
    mask = perm >= 0
    slots[mask] = np.asarray(x, np.float32)[perm[mask]]
    # [core, 64, NPC]
    xT = np.ascontiguousarray(
        slots.reshape(NCORES, cfg.NPC, D).transpose(0, 2, 1))

    # ---- edges routed to windows, padded to CAP per window
    eslot = slot_of_node[row]
    ewin = eslot // W
    erel = (eslot % W).astype(np.float32)
    eorder = np.argsort(ewin, kind="stable")
    ewin_s = ewin[eorder]
    counts = np.bincount(ewin, minlength=cfg.NWIN)
    starts = np.concatenate([[0], np.cumsum(counts)[:-1]])
    rank = np.arange(n_edges) - starts[ewin_s]

    estream = np.zeros((cfg.NWIN, CAP, D), ml_dtypes.bfloat16)
    rstream = np.zeros((cfg.NWIN, CAP), ml_dtypes.bfloat16)
    estream[ewin_s, rank] = np.asarray(edge_attr, np.float32)[eorder].astype(
        ml_dtypes.bfloat16)
    rstream[ewin_s, rank] = erel[eorder].astype(ml_dtypes.bfloat16)

    # device edge layout: [core, NGRP, 128, G*CHUNKS, 64]
    # within a window, edge slot e = 8*p + c  ->  tile[p, (wg*8+c), :]
    e5 = estream.reshape(NCORES, cfg.NGRP, G, CHUNK, CHUNKS, D)
    edges = np.ascontiguousarray(e5.transpose(0, 1, 3, 2, 4, 5)).reshape(
        NCORES, cfg.NGRP, CHUNK, G * CHUNKS, D)
    r3 = rstream.reshape(NCORES, cfg.WPC, CHUNK, CHUNKS)
    rels = np.ascontiguousarray(r3.transpose(0, 2, 1, 3)).reshape(
        NCORES, CHUNK, cfg.WPC * CHUNKS)

    iota = np.ascontiguousarray(
        np.tile(np.arange(W, dtype=ml_dtypes.bfloat16), (CHUNK, G * CHUNKS, 1)))

    # catT on device holds agg rows on partitions 0:64 and x on 64:128, so
    # swap W1's row halves to match: rows 0:64 must weight agg features.
    W1f = np.asarray(W1, np.float32)
    W1p = np.ascontiguousarray(
        np.concatenate([W1f[D:2 * D], W1f[0:D]], axis=0))  # [128, 256]
    W2p = np.ascontiguousarray(
        np.asarray(W2, np.float32).reshape(2, 128, O).transpose(1, 0, 2)
        .reshape(128, 2 * O))                              # [128, 128]
    b1T = np.ascontiguousarray(
        np.asarray(b1, np.float32).reshape(2, 128).T)      # [128, 2]
    b2c = np.asarray(b2, np.float32).reshape(O, 1)         # [64, 1]

    in_maps = []
    for c in range(NCORES):
        in_maps.append({
            "xT": xT[c], "edges": edges[c], "rels": rels[c], "iota": iota,
            "W1": W1p, "W2p": W2p, "b1T": b1T, "b2": b2c,
        })
    return in_maps, perm, mask


# -------------------------------------------------------------- device build

def build_nc(cfg):
    nc = bacc.Bacc("TRN2", target_bir_lowering=False, debug=False)
    ap_xT = nc.dram_tensor("xT", [D, cfg.NPC], F32R, kind="ExternalInput").ap()
    ap_edges = nc.dram_tensor(
        "edges", [cfg.NGRP, CHUNK, G * CHUNKS, D], BF16,
        kind="ExternalInput").ap()
    ap_rels = nc.dram_tensor(
        "rels", [CHUNK, cfg.WPC * CHUNKS], BF16, kind="ExternalInput").ap()
    ap_iota = nc.dram_tensor(
        "iota", [CHUNK, G * CHUNKS, W], BF16, kind="ExternalInput").ap()
    ap_W1 = nc.dram_tensor("W1", [2 * D, H], F32R, kind="ExternalInput").ap()
    ap_W2p = nc.dram_tensor("W2p", [H // 2, 2 * O], F32R,
                            kind="ExternalInput").ap()
    ap_b1T = nc.dram_tensor("b1T", [H // 2, 2], F32, kind="ExternalInput").ap()
    ap_b2 = nc.dram_tensor("b2", [O, 1], F32, kind="ExternalInput").ap()
    ap_out = nc.dram_tensor("outT", [O, cfg.NPC], F32,
                            kind="ExternalOutput").ap()

    AF = mybir.ActivationFunctionType
    with tile.TileContext(nc) as tc, ExitStack() as ctx:
        consts = ctx.enter_context(tc.tile_pool(name="consts", bufs=1))
        epool = ctx.enter_context(tc.tile_pool(name="edges", bufs=3))
        opool = ctx.enter_context(tc.tile_pool(name="onehot", bufs=3))
        hpool = ctx.enter_context(tc.tile_pool(name="hid", bufs=3))
        ypool = ctx.enter_context(tc.tile_pool(name="yout", bufs=2))
        ps_a = ctx.enter_context(tc.tile_pool(name="ps_agg", bufs=2,
                                              space="PSUM"))
        ps_h = ctx.enter_context(tc.tile_pool(name="ps_h", bufs=3,
                                              space="PSUM"))
        ps_o = ctx.enter_context(tc.tile_pool(name="ps_o", bufs=2,
                                              space="PSUM"))

        # catT: partitions 0:64 = aggT (written per window), 64:128 = xT
        catT = consts.tile([2 * D, cfg.NPC], F32R)
        nc.sync.dma_start(catT[D:2 * D, :], ap_xT)
        rels = consts.tile([CHUNK, cfg.WPC * CHUNKS], BF16)
        nc.sync.dma_start(rels[:], ap_rels)
        iota = consts.tile([CHUNK, G * CHUNKS, W], BF16)
        nc.sync.dma_start(iota[:], ap_iota)
        W1t = consts.tile([2 * D, H], F32R)
        nc.sync.dma_start(W1t[:], ap_W1)
        W2t = consts.tile([H // 2, 2 * O], F32R)
        nc.sync.dma_start(W2t[:], ap_W2p)
        b1T = consts.tile([H // 2, 2], F32)
        nc.sync.dma_start(b1T[:], ap_b1T)
        b2t = consts.tile([O, 1], F32)
        nc.sync.dma_start(b2t[:], ap_b2)

        def mlp(st):
            cat_sl = catT[:, st * ST:(st + 1) * ST]
            hs = []
            for half in range(2):
                w1h = W1t[:, half * 128:(half + 1) * 128]
                h_ps = ps_h.tile([128, ST], F32, tag="h_ps")
                nc.tensor.matmul(h_ps[:], w1h, cat_sl, start=True, stop=True)
                h_sb = hpool.tile([128, ST], F32R, tag="h_sb")
                nc.scalar.activation(h_sb[:], h_ps[:], AF.Relu,
                                     bias=b1T[:, half:half + 1])
                hs.append(h_sb)
            o_ps = ps_o.tile([O, ST], F32)
            nc.tensor.matmul(o_ps[:], W2t[:, 0:O], hs[0][:],
                             start=True, stop=False)
            nc.tensor.matmul(o_ps[:], W2t[:, O:2 * O], hs[1][:],
                             start=False, stop=True)
            o_sb = ypool.tile([O, ST], F32)
            nc.scalar.activation(o_sb[:], o_ps[:], AF.Identity, bias=b2t[:])
            nc.sync.dma_start(ap_out[:, st * ST:(st + 1) * ST], o_sb[:])

        for g in range(cfg.NGRP):
            et = epool.tile([CHUNK, G * CHUNKS, D], BF16)
            nc.sync.dma_start(et[:], ap_edges[g])
            oh = opool.tile([CHUNK, G * CHUNKS, W], BF16)
            rel_bc = (rels[:, g * G * CHUNKS:(g + 1) * G * CHUNKS]
                      .unsqueeze(2).broadcast_to([CHUNK, G * CHUNKS, W]))
            nc.vector.tensor_tensor(oh[:], iota[:], rel_bc,
                                    op=mybir.AluOpType.is_equal)
            for wg in range(G):
                win = g * G + wg
                a_ps = ps_a.tile([D, W], F32)
                for c in range(CHUNKS):
                    cc = wg * CHUNKS + c
                    nc.tensor.matmul(a_ps[:], et[:, cc, :], oh[:, cc, :],
                                     start=(c == 0), stop=(c == CHUNKS - 1))
                nc.scalar.activation(catT[0:D, win * W:(win + 1) * W],
                                     a_ps[:], AF.Copy)
            if g % 2 == 1:
                mlp(g // 2)
    nc.compile()
    return nc


# ------------------------------------------------------------------- driver

_CACHE = {}


def prepare(inputs):
    x = np.asarray(inputs["x"])
    edge_index = np.asarray(inputs["edge_index"])
    edge_attr = np.asarray(inputs["edge_attr"])
    cfg = Cfg(x.shape[0], edge_attr.shape[0])
    in_maps, perm, mask = _pack(
        x, edge_index, edge_attr,
        inputs["W1"], inputs["b1"], inputs["W2"], inputs["b2"], cfg)
    key = (x.shape[0], edge_attr.shape[0])
    if key not in _CACHE:
        _CACHE[key] = build_nc(cfg)
    return _CACHE[key], in_maps, cfg, perm, mask


def unpack_out(results, cfg, perm, mask, n_nodes):
    slots = np.concatenate(
        [np.asarray(r["outT"], np.float32).T for r in results], axis=0)
    y = np.zeros((n_nodes, O), np.float32)
    y[perm[mask]] = slots[mask]
    return y


def kernel(**inputs):
    nc, in_maps, cfg, perm, mask = prepare(inputs)
    res = run_bass_kernel_spmd(nc, in_maps, list(range(NCORES)))
    return unpack_out(res.results, cfg, perm, mask,
                      np.asarray(inputs["x"]).shape[0])


# revision 8
# speedup vs baseline: 1.1353x; 1.1353x over previous
"""Trainium2 Bass kernel for nn_NodeModel (gnn_message_passing).

Reference computation:
    agg = segment_sum(edge_attr, edge_index[0], N)   # [N, 64]
    h   = relu(concat([x, agg], 1) @ W1 + b1)        # [N, 256]
    out = h @ W2 + b2                                # [N, 64]
(u and batch are unused by the reference.)

Strategy (8 cores, graph-parallel):
  * Host assigns nodes to 32-node "windows" (400 windows/core), balancing
    edge counts per window with a greedy heap so every window holds <= 512
    edges.  Edges are routed to the window owning their row endpoint and
    padded to exactly 4 chunks of 128 edges per window.
  * Device scatter-add: for each 128-edge chunk, build a one-hot
    [128 edges, 32 nodes] matrix on the Vector engine (iota == rel compare)
    and accumulate  aggT[64f, 32n] += edge_chunk.T @ onehot  on the PE.
    16 windows share one [64, 512] PSUM supertile; a single ACT copy moves
    it into SBUF.  Everything stays feature-major so no transposes exist.
  * MLP runs on the same 512-node supertiles:
    hT = relu(W1.T @ [aggT; xT] + b1), outT = W2.T @ hT + b2.
  * Edge data travels as bf16 (halves the dominant DMA traffic) via the
    SWDGE/gpsimd path; the MLP runs in float32r (fast PE mode, ~1e-4).
"""

import os
import sys
import heapq

for _p in ("/opt/trn_rl_repo", "/root/.axon_site/_ro/trn_rl_repo"):
    if os.path.isdir(_p) and _p not in sys.path:
        sys.path.insert(0, _p)

import numpy as np
import ml_dtypes
from contextlib import ExitStack

import concourse.bass as bass
import concourse.tile as tile
from concourse import bacc, mybir
from concourse.bass_utils import run_bass_kernel_spmd

F32 = mybir.dt.float32
F32R = mybir.dt.float32r
BF16 = mybir.dt.bfloat16

NCORES = 8
D = 64            # feature dim
H = 256           # hidden dim
O = 64            # output dim
W = 32            # nodes per window
CHUNK = 128       # edges per chunk (PE contraction dim)
CHUNKS = 4        # chunks per window
CAP = CHUNK * CHUNKS   # 512 edge slots per window
G = 16            # windows per group (= one supertile, one edge DMA)
ST = G * W        # 512-node MLP supertile


class Cfg:
    def __init__(self, n_nodes, n_edges, extra=0):
        wpc = max(
            (n_nodes + NCORES * W - 1) // (NCORES * W),
            int(np.ceil(n_edges * 1.02 / (CAP * NCORES))),
        ) + extra
        wpc = ((wpc + G - 1) // G) * G            # supertile-align
        self.WPC = wpc                            # windows per core
        self.NPC = W * wpc                        # node slots per core
        self.NWIN = NCORES * wpc
        self.NGRP = wpc // G                      # groups (= supertiles)


# ----------------------------------------------------------------- host pack

class PackOverflow(Exception):
    pass


def _assign_nodes(row, n_nodes, cfg):
    """slot_of_node: balanced node->slot map; perm: slot->node (-1 = pad)."""
    deg = np.bincount(row, minlength=n_nodes)
    order = np.argsort(-deg, kind="stable")
    nwin = cfg.NWIN
    heap = [(0, w) for w in range(nwin)]
    counts = np.zeros(nwin, np.int64)     # nodes per window
    loads = np.zeros(nwin, np.int64)      # edges per window
    slot_of_node = np.full(n_nodes, -1, np.int64)
    for n in order:
        d = int(deg[n])
        while True:
            load, w = heapq.heappop(heap)
            if counts[w] < W:
                break
        slot_of_node[n] = w * W + counts[w]
        counts[w] += 1
        loads[w] = load + d
        if counts[w] < W:
            heapq.heappush(heap, (loads[w], w))
    if loads.max() > CAP:
        raise PackOverflow(f"window overflow: {loads.max()} > {CAP}")
    perm = np.full(nwin * W, -1, np.int64)
    perm[slot_of_node] = np.arange(n_nodes)
    return slot_of_node, perm


def _pack(x, edge_index, edge_attr, W1, b1, W2, b2, cfg):
    n_nodes = x.shape[0]
    n_edges = edge_attr.shape[0]
    row = np.asarray(edge_index[0], np.int64)
    slot_of_node, perm = _assign_nodes(row, n_nodes, cfg)

    # ---- node features, transposed + permuted, split per core
    slots = np.zeros((cfg.NWIN * W, D), np.float32)
    mask = perm >= 0
    slots[mask] = np.asarray(x, np.float32)[perm[mask]]
    xT = np.ascontiguousarray(
        slots.reshape(NCORES, cfg.NPC, D).transpose(0, 2, 1))

    # ---- edges routed to windows, padded to CAP per window
    eslot = slot_of_node[row]
    ewin = eslot // W
    erel = (eslot % W).astype(np.float32)
    eorder = np.argsort(ewin, kind="stable")
    ewin_s = ewin[eorder]
    counts = np.bincount(ewin, minlength=cfg.NWIN)
    starts = np.concatenate([[0], np.cumsum(counts)[:-1]])
    rank = np.arange(n_edges) - starts[ewin_s]

    estream = np.zeros((cfg.NWIN, CAP, D), ml_dtypes.bfloat16)
    rstream = np.zeros((cfg.NWIN, CAP), ml_dtypes.bfloat16)
    estream[ewin_s, rank] = np.asarray(edge_attr, np.float32)[eorder].astype(
        ml_dtypes.bfloat16)
    rstream[ewin_s, rank] = erel[eorder].astype(ml_dtypes.bfloat16)

    # device edge layout: [core, NGRP, 128, G*CHUNKS, 64]
    # within a window, edge slot e = CHUNKS*p + c -> tile[p, (wg*CHUNKS+c), :]
    e6 = estream.reshape(NCORES, cfg.NGRP, G, CHUNK, CHUNKS, D)
    edges = np.ascontiguousarray(e6.transpose(0, 1, 3, 2, 4, 5)).reshape(
        NCORES, cfg.NGRP, CHUNK, G * CHUNKS, D)
    r4 = rstream.reshape(NCORES, cfg.WPC, CHUNK, CHUNKS)
    rels = np.ascontiguousarray(r4.transpose(0, 2, 1, 3)).reshape(
        NCORES, CHUNK, cfg.WPC * CHUNKS)

    iota = np.ascontiguousarray(
        np.tile(np.arange(W, dtype=ml_dtypes.bfloat16),
                (CHUNK, G * CHUNKS, 1)))

    # catT on device holds agg rows on partitions 0:64 and x on 64:128, so
    # swap W1's row halves to match: rows 0:64 must weight agg features.
    W1f = np.asarray(W1, np.float32)
    W1p = np.ascontiguousarray(
        np.concatenate([W1f[D:2 * D], W1f[0:D]], axis=0))  # [128, 256]
    W2p = np.ascontiguousarray(
        np.asarray(W2, np.float32).reshape(2, 128, O).transpose(1, 0, 2)
        .reshape(128, 2 * O))                              # [128, 128]
    b1T = np.ascontiguousarray(
        np.asarray(b1, np.float32).reshape(2, 128).T)      # [128, 2]
    b2c = np.asarray(b2, np.float32).reshape(O, 1)         # [64, 1]

    in_maps = []
    for c in range(NCORES):
        in_maps.append({
            "xT": xT[c], "edges": edges[c], "rels": rels[c], "iota": iota,
            "W1": W1p, "W2p": W2p, "b1T": b1T, "b2": b2c,
        })
    return in_maps, perm, mask


# -------------------------------------------------------------- device build

def build_nc(cfg, reps=1):
    nc = bacc.Bacc("TRN2", target_bir_lowering=False, debug=False)
    ap_xT = nc.dram_tensor("xT", [D, cfg.NPC], F32R,
                           kind="ExternalInput").ap()
    ap_edges = nc.dram_tensor(
        "edges", [cfg.NGRP, CHUNK, G * CHUNKS, D], BF16,
        kind="ExternalInput").ap()
    ap_rels = nc.dram_tensor(
        "rels", [CHUNK, cfg.WPC * CHUNKS], BF16, kind="ExternalInput").ap()
    ap_iota = nc.dram_tensor(
        "iota", [CHUNK, G * CHUNKS, W], BF16, kind="ExternalInput").ap()
    ap_W1 = nc.dram_tensor("W1", [2 * D, H], F32R, kind="ExternalInput").ap()
    ap_W2p = nc.dram_tensor("W2p", [H // 2, 2 * O], F32R,
                            kind="ExternalInput").ap()
    ap_b1T = nc.dram_tensor("b1T", [H // 2, 2], F32,
                            kind="ExternalInput").ap()
    ap_b2 = nc.dram_tensor("b2", [O, 1], F32, kind="ExternalInput").ap()
    ap_out = nc.dram_tensor("outT", [O, cfg.NPC], F32,
                            kind="ExternalOutput").ap()

    AF = mybir.ActivationFunctionType
    with tile.TileContext(nc) as tc, ExitStack() as ctx:
        consts = ctx.enter_context(tc.tile_pool(name="consts", bufs=1))
        epool = ctx.enter_context(tc.tile_pool(name="edges", bufs=3))
        opool = ctx.enter_context(tc.tile_pool(name="onehot", bufs=3))
        hpool = ctx.enter_context(tc.tile_pool(name="hid", bufs=3))
        ypool = ctx.enter_context(tc.tile_pool(name="yout", bufs=2))
        ps_a = ctx.enter_context(tc.tile_pool(name="ps_agg", bufs=2,
                                              space="PSUM"))
        ps_h = ctx.enter_context(tc.tile_pool(name="ps_h", bufs=3,
                                              space="PSUM"))
        ps_o = ctx.enter_context(tc.tile_pool(name="ps_o", bufs=2,
                                              space="PSUM"))

        # catT: partitions 0:64 = aggT (written per supertile), 64:128 = xT
        catT = consts.tile([2 * D, cfg.NPC], F32R)
        nc.sync.dma_start(catT[D:2 * D, :], ap_xT)
        rels = consts.tile([CHUNK, cfg.WPC * CHUNKS], BF16)
        nc.sync.dma_start(rels[:], ap_rels)
        iota = consts.tile([CHUNK, G * CHUNKS, W], BF16)
        nc.sync.dma_start(iota[:], ap_iota)
        W1t = consts.tile([2 * D, H], F32R)
        nc.sync.dma_start(W1t[:], ap_W1)
        W2t = consts.tile([H // 2, 2 * O], F32R)
        nc.sync.dma_start(W2t[:], ap_W2p)
        b1T = consts.tile([H // 2, 2], F32)
        nc.sync.dma_start(b1T[:], ap_b1T)
        b2t = consts.tile([O, 1], F32)
        nc.sync.dma_start(b2t[:], ap_b2)

        def mlp(st):
            cat_sl = catT[:, st * ST:(st + 1) * ST]
            hs = []
            for half in range(2):
                w1h = W1t[:, half * 128:(half + 1) * 128]
                h_ps = ps_h.tile([128, ST], F32, tag="h_ps")
                nc.tensor.matmul(h_ps[:], w1h, cat_sl, start=True, stop=True)
                h_sb = hpool.tile([128, ST], F32R, tag="h_sb")
                nc.scalar.activation(h_sb[:], h_ps[:], AF.Relu,
                                     bias=b1T[:, half:half + 1])
                hs.append(h_sb)
            o_ps = ps_o.tile([O, ST], F32)
            nc.tensor.matmul(o_ps[:], W2t[:, 0:O], hs[0][:],
                             start=True, stop=False)
            nc.tensor.matmul(o_ps[:], W2t[:, O:2 * O], hs[1][:],
                             start=False, stop=True)
            o_sb = ypool.tile([O, ST], F32)
            nc.scalar.activation(o_sb[:], o_ps[:], AF.Identity, bias=b2t[:])
            nc.sync.dma_start(ap_out[:, st * ST:(st + 1) * ST], o_sb[:])

        for rep in range(reps):
            for g in range(cfg.NGRP):
                et = epool.tile([CHUNK, G * CHUNKS, D], BF16)
                nc.gpsimd.dma_start(et[:], ap_edges[g])
                oh = opool.tile([CHUNK, G * CHUNKS, W], BF16)
                rel_bc = (rels[:, g * G * CHUNKS:(g + 1) * G * CHUNKS]
                          .unsqueeze(2).broadcast_to([CHUNK, G * CHUNKS, W]))
                nc.vector.tensor_tensor(oh[:], iota[:], rel_bc,
                                        op=mybir.AluOpType.is_equal)
                a_ps = ps_a.tile([D, ST], F32)
                for wg in range(G):
                    for c in range(CHUNKS):
                        cc = wg * CHUNKS + c
                        nc.tensor.matmul(a_ps[:, wg * W:(wg + 1) * W],
                                         et[:, cc, :], oh[:, cc, :],
                                         start=(c == 0),
                                         stop=(c == CHUNKS - 1))
                nc.scalar.activation(catT[0:D, g * ST:(g + 1) * ST],
                                     a_ps[:], AF.Copy)
                mlp(g)
    nc.compile()
    return nc


# ------------------------------------------------------------------- driver

_CACHE = {}


def prepare(inputs, reps=1):
    x = np.asarray(inputs["x"])
    edge_index = np.asarray(inputs["edge_index"])
    edge_attr = np.asarray(inputs["edge_attr"])
    for extra in (0, 16, 32, 64):
        cfg = Cfg(x.shape[0], edge_attr.shape[0], extra=extra)
        try:
            in_maps, perm, mask = _pack(
                x, edge_index, edge_attr,
                inputs["W1"], inputs["b1"], inputs["W2"], inputs["b2"], cfg)
            break
        except PackOverflow:
            continue
    else:
        raise RuntimeError("could not pack edges into windows")
    key = (cfg.WPC, reps)
    if key not in _CACHE:
        _CACHE[key] = build_nc(cfg, reps=reps)
    return _CACHE[key], in_maps, cfg, perm, mask


def unpack_out(results, cfg, perm, mask, n_nodes):
    slots = np.concatenate(
        [np.asarray(r["outT"], np.float32).T for r in results], axis=0)
    y = np.zeros((n_nodes, O), np.float32)
    y[perm[mask]] = slots[mask]
    return y


def kernel(**inputs):
    nc, in_maps, cfg, perm, mask = prepare(inputs)
    res = run_bass_kernel_spmd(nc, in_maps, list(range(NCORES)))
    return unpack_out(res.results, cfg, perm, mask,
                      np.asarray(inputs["x"]).shape[0])
